# revision 16
# baseline (speedup 1.0000x reference)
"""CoAttention kernel v13 for 8 TRN2 NeuronCores.

Sharding: batch (4) x role (2) = 8 cores, no collectives (role symmetry:
role E computes A^T via swapped inputs, so both roles run one program).

v8 vs v7: phase-split main loop driven by HW micro-benchmarks
(distinct-moving bf16 N=512 MM floor ~250-270ns; ACT exp = (N+352)/1.2;
group boundaries, not LDWEIGHTS, dominate per-MM overhead):
  - A-phase per mj: 16 nj-pairs, 4 MMs each with the two accumulation
    groups bank-interleaved so consecutive MMs stream the same y half
    (moving-operand reuse), PSUM pair tiles [128,2,512] spanning 2 banks.
  - One packed ACT exp per nj-pair over both banks ((1024+352)/1.2 vs
    2x(512+352)/1.2) -> ACT 18.3us/mj, below the PE phase time.
  - U-phase per mj: two uninterrupted 32-long accumulation chains
    (u_ps0/u_ps1), consuming P tiles in exp-production order so the
    chain never waits on ACT.
  - Tail groups of mj are slotted into mj+1's A-phase (cross-engine
    latency hides under PE work), as in v7.

Per-core program (C=256, n pixels stationary-side, m pixels moving-side):
  EC = Wh @ X                      [C, n]  (bf16)
  for each m-chunk (512):
    A-phase: A_pair = EC^T @ Y     (PE, 2-bank pairs) -> exp -> P (bf16)
             cs += P               (DVE)
    U-phase: U = X @ P             (PE, 2x32 chains)
    tail:    colsum (PE ones), recip, gate dot, sigmoid scale,
             broadcast (PE outer), gated mul, out = WaT^T@gated + WbT^T@Y
"""

import numpy as np
import ml_dtypes

import concourse.bass as bass
import concourse.bacc as bacc
import concourse.tile as tile
from concourse import mybir
from concourse import bass_utils

F32 = mybir.dt.float32
F32R = mybir.dt.float32r
BF16 = mybir.dt.bfloat16

B = 4
C = 256
H = 64
W = 64
HW = H * W
KEXP = 20.0  # constant subtracted before exp (softmax-invariant)

TRACE = False
AHEADP = 2  # U-pair(njp-AHEADP) is emitted after A-pair(njp)

_COMPILED = {}


def _build_nc(n_pix, m_pix, rep=1):
    nc = bacc.Bacc(
        "TRN2",
        target_bir_lowering=False,
        debug=False,
        enable_asserts=True,
        num_devices=8,
    )
    X = nc.dram_tensor("x", [C, n_pix], BF16, kind="ExternalInput").ap()
    XT = nc.dram_tensor("xt", [n_pix, C], BF16, kind="ExternalInput").ap()
    Y = nc.dram_tensor("y", [C, m_pix], BF16, kind="ExternalInput").ap()
    WHT = nc.dram_tensor("wht", [C, C], BF16, kind="ExternalInput").ap()
    WAT = nc.dram_tensor("wat", [C, C], BF16, kind="ExternalInput").ap()
    WBT = nc.dram_tensor("wbt", [C, C], BF16, kind="ExternalInput").ap()
    GW = nc.dram_tensor("gw", [C, 1], BF16, kind="ExternalInput").ap()
    ONESC = nc.dram_tensor("onescol", [128, 1], BF16, kind="ExternalInput").ap()
    ONESR = nc.dram_tensor("onesrow", [1, 128], F32R, kind="ExternalInput").ap()
    OUT = nc.dram_tensor("out", [C, m_pix], F32, kind="ExternalOutput").ap()

    NCH = n_pix // 128   # 128-row n chunks (32)
    NPR = NCH // 2       # nj pairs per m-chunk (16)
    MCH = m_pix // 512   # 512-wide m chunks (8)
    NK = n_pix // 512    # 512-wide n chunks for the EC phase
    Exp = mybir.ActivationFunctionType.Exp
    Tanh = mybir.ActivationFunctionType.Tanh
    Copy = mybir.ActivationFunctionType.Copy

    with tile.TileContext(nc) as tc:
        with (
            nc.allow_low_precision(reason="bf16 matmul operands"),
            tc.tile_pool(name="persist", bufs=1) as persist,
            tc.tile_pool(name="psA", bufs=2, space=bass.MemorySpace.PSUM) as psA,
            tc.tile_pool(name="psU", bufs=1, space=bass.MemorySpace.PSUM) as psU,
            tc.tile_pool(name="psO", bufs=2, space=bass.MemorySpace.PSUM) as psO,
            tc.tile_pool(name="ppool", bufs=8) as ppool,
            tc.tile_pool(name="accp", bufs=2) as accp,
            tc.tile_pool(name="upool", bufs=2) as upool,
            tc.tile_pool(name="opool", bufs=2) as opool,
            tc.tile_pool(name="small", bufs=2) as small,
        ):
            # ---- persistent loads, ordered+chunked by first consumption ----
            Xr = X.rearrange("(ci p) n -> p ci n", p=128)
            Yr = Y.rearrange("(ci p) m -> p ci m", p=128)
            XTr = XT.rearrange("(a p) c -> p a c", p=128)
            wht_sb = persist.tile([128, 2, C], BF16)
            nc.sync.dma_start(out=wht_sb, in_=WHT.rearrange("(ci p) d -> p ci d", p=128))
            ones_col = persist.tile([128, 1], BF16)
            nc.sync.dma_start(out=ones_col, in_=ONESC)
            ones_row = persist.tile([1, 128], F32R)
            nc.sync.dma_start(out=ones_row, in_=ONESR)
            x_sb = persist.tile([128, 2, n_pix], BF16)
            for ci in range(2):
                nc.sync.dma_start(out=x_sb[:, ci, 0:512], in_=Xr[:, ci, 0:512])
            for ci in range(2):
                nc.sync.dma_start(out=x_sb[:, ci, 512:1024], in_=Xr[:, ci, 512:1024])
            for nk in range(1, NK // 2):
                nsl = slice(nk * 1024, (nk + 1) * 1024)
                for ci in range(2):
                    nc.sync.dma_start(out=x_sb[:, ci, nsl], in_=Xr[:, ci, nsl])
            y_sb = persist.tile([128, 2, m_pix], BF16)
            for ci in range(2):
                nc.sync.dma_start(out=y_sb[:, ci, 0:512], in_=Yr[:, ci, 0:512])
            xT_sb = persist.tile([128, NCH, C], BF16)
            for a in range(0, NCH, 4):
                nc.sync.dma_start(out=xT_sb[:, a:a + 4, :], in_=XTr[:, a:a + 4, :])
            for mk in range(1, MCH):
                msl_ = slice(mk * 512, (mk + 1) * 512)
                for ci in range(2):
                    nc.sync.dma_start(out=y_sb[:, ci, msl_], in_=Yr[:, ci, msl_])
            wat_sb = persist.tile([128, 2, C], BF16)
            nc.sync.dma_start(out=wat_sb, in_=WAT.rearrange("(ci p) o -> p ci o", p=128))
            wbt_sb = persist.tile([128, 2, C], BF16)
            nc.sync.dma_start(out=wbt_sb, in_=WBT.rearrange("(ci p) o -> p ci o", p=128))
            gw_sb = persist.tile([128, 2, 1], BF16)
            nc.sync.dma_start(out=gw_sb, in_=GW.rearrange("(ci p) o -> p ci o", p=128))
            negk128 = persist.tile([128, 1], F32)
            nc.vector.memset(negk128, -KEXP)
            zero1 = persist.tile([1, 1], F32)
            nc.vector.memset(zero1, 0.0)
            ec_sb = persist.tile([128, 2, n_pix], BF16)

            # ---- EC = Wh @ X (bf16), nk-major, emitted interleaved with the
            # first A-phase so PE fills the x-DMA stream gaps ----
            def emit_ec(nk):
                nsl = slice(nk * 512, (nk + 1) * 512)
                ec_ps = psA.tile([128, 2, 512], F32, tag="a")
                # consecutive MMs share moving x half; banks hold dj halves
                for ci in range(2):
                    for dj in range(2):
                        nc.tensor.matmul(
                            ec_ps[:, dj, :],
                            wht_sb[:, ci, dj * 128:(dj + 1) * 128],
                            x_sb[:, ci, nsl],
                            start=(ci == 0),
                            stop=(ci == 1),
                        )
                nc.scalar.activation(ec_sb[:, :, nsl], ec_ps, Copy)

            # ---- main loop: per mj an A-phase then a U-phase; tail of mj
            # interleaved into mj+1's A-phase at these nj-pair slots ----
            TAIL_SLOTS = {3: 0, 6: 1, 9: 2, 12: 3, 15: 4}

            def make_tail(msl, u_ps0, u_ps1, cs_parts):
                st = {}

                def g0():  # colsum -> recip; copy U out of PSUM (bf16)
                    cs_ps = psO.tile([1, 512], F32, tag="o")
                    nc.tensor.matmul(cs_ps, ones_col, cs_parts[0],
                                     start=True, stop=False)
                    nc.tensor.matmul(cs_ps, ones_col, cs_parts[1],
                                     start=False, stop=True)
                    st["recip"] = small.tile([1, 512], F32R, tag="recip", name="recip")
                    nc.vector.reciprocal(st["recip"], cs_ps)
                    st["u_sb0"] = upool.tile([128, 512], BF16, tag="usb0", name="usb0")
                    st["u_sb1"] = upool.tile([128, 512], BF16, tag="usb1", name="usb1")
                    nc.vector.tensor_copy(st["u_sb0"], u_ps0)
                    nc.vector.tensor_copy(st["u_sb1"], u_ps1)

                def g1():  # gate dot product
                    st["gd_ps"] = psO.tile([1, 512], F32, tag="o", name="gdps")
                    nc.tensor.matmul(st["gd_ps"], gw_sb[:, 0, :], st["u_sb0"],
                                     start=True, stop=False)
                    nc.tensor.matmul(st["gd_ps"], gw_sb[:, 1, :], st["u_sb1"],
                                     start=False, stop=True)

                def g2():  # scale = sigmoid(gdot/colsum)/colsum; bcast; gated
                    t_sb = small.tile([1, 512], F32R, tag="t")
                    nc.vector.tensor_mul(t_sb, st["gd_ps"], st["recip"])
                    # sigmoid via tanh (same ACT table-set as Exp):
                    # sigmoid(t) = (1+tanh(t/2))/2; the 1/2 is folded into
                    # the host-provided broadcast row (0.5 instead of 1.0)
                    th_sb = small.tile([1, 512], F32, tag="e")
                    nc.scalar.activation(th_sb, t_sb, Tanh, bias=zero1, scale=0.5)
                    scale_sb = small.tile([1, 512], F32R, tag="scale")
                    nc.vector.scalar_tensor_tensor(
                        scale_sb, th_sb, 1.0, st["recip"],
                        mybir.AluOpType.add, mybir.AluOpType.mult)
                    bc_ps = psO.tile([128, 512], F32, tag="o")
                    nc.tensor.matmul(bc_ps, ones_row, scale_sb)
                    st["gated0"] = upool.tile([128, 512], BF16, tag="gated0", name="gated0")
                    st["gated1"] = upool.tile([128, 512], BF16, tag="gated1", name="gated1")
                    nc.vector.tensor_mul(st["gated0"], st["u_sb0"], bc_ps)
                    nc.vector.tensor_mul(st["gated1"], st["u_sb1"], bc_ps)

                def out_conv(oj):
                    osl = slice(oj * 128, (oj + 1) * 128)
                    o_ps = psO.tile([128, 512], F32, tag="o")
                    gated = [st["gated0"], st["gated1"]]
                    # y-term first: independent of the gate chain, so these
                    # MMs issue while sigmoid/scale are still in flight
                    for ci in range(2):
                        nc.tensor.matmul(o_ps, wbt_sb[:, ci, osl], y_sb[:, ci, msl],
                                         start=(ci == 0), stop=False)
                    for ci in range(2):
                        nc.tensor.matmul(o_ps, wat_sb[:, ci, osl], gated[ci],
                                         start=False, stop=(ci == 1))
                    o_sb = opool.tile([128, 512], F32, tag="osb")
                    nc.vector.tensor_copy(o_sb, o_ps)
                    nc.sync.dma_start(out=OUT[osl, msl], in_=o_sb)

                return [g0, g1, g2, lambda: out_conv(0), lambda: out_conv(1)]

            pending = None
            for nk in range(NK - 2):
                emit_ec(nk)
            ec_defer = [NK - 2, NK - 1]
            mjs = [mj for _ in range(rep) for mj in range(MCH)]
            for mji, mj in enumerate(mjs):
                is_last = mji == len(mjs) - 1
                msl = slice(mj * 512, (mj + 1) * 512)
                cs_parts = [accp.tile([128, 512], BF16, tag="cs0", name="cs0"),
                            accp.tile([128, 512], BF16, tag="cs1", name="cs1")]
                u_ps0 = psU.tile([128, 512], F32, tag="u0")
                u_ps1 = psU.tile([128, 512], F32, tag="u1")
                p_tiles = []

                def emit_u_pair(njp):
                    for nj in (2 * njp, 2 * njp + 1):
                        pj = p_tiles[nj // 2][:, nj % 2, :]
                        nc.tensor.matmul(u_ps0, xT_sb[:, nj, 0:128], pj,
                                         start=(nj == 0), stop=(nj == NCH - 1))
                        if not is_last:
                            nc.tensor.matmul(u_ps1, xT_sb[:, nj, 128:256], pj,
                                             start=(nj == 0), stop=(nj == NCH - 1))

                # ---- 16 nj-pairs: A-pair, packed exp, cs, U-pair (lagged) ----
                for njp in range(NPR):
                    # mj0's A-phase is otherwise ACT-paced (no tail/U work
                    # yet): fill PE with the deferred EC chunks
                    if ec_defer and njp in (1, 3):
                        emit_ec(ec_defer.pop(0))
                    n0 = slice((2 * njp) * 128, (2 * njp + 1) * 128)
                    n1 = slice((2 * njp + 1) * 128, (2 * njp + 2) * 128)
                    a_ps = psA.tile([128, 2, 512], F32, tag="a")
                    # bank-interleaved groups; consecutive MMs share moving y
                    nc.tensor.matmul(a_ps[:, 0, :], ec_sb[:, 0, n0],
                                     y_sb[:, 0, msl], start=True, stop=False)
                    nc.tensor.matmul(a_ps[:, 1, :], ec_sb[:, 0, n1],
                                     y_sb[:, 0, msl], start=True, stop=False)
                    nc.tensor.matmul(a_ps[:, 0, :], ec_sb[:, 1, n0],
                                     y_sb[:, 1, msl], start=False, stop=True)
                    nc.tensor.matmul(a_ps[:, 1, :], ec_sb[:, 1, n1],
                                     y_sb[:, 1, msl], start=False, stop=True)
                    p_sb = ppool.tile([128, 2, 512], BF16, tag="p", name="p")
                    p_tiles.append(p_sb)
                    # one packed exp over both banks
                    nc.scalar.activation(p_sb, a_ps, Exp, bias=negk128, scale=1.0)
                    # bf16 pair-sum, then all-bf16 partial accumulate
                    # (both 2x-packed DVE mode; no f32 accumulator rw).
                    # Cuts DVE SBUF traffic ~45% vs per-tile f32 adds,
                    # relieving read-port contention with the PE moving
                    # stream; colsum sums the two bf16 partials on PE.
                    ptmp = small.tile([128, 512], BF16, tag="ptmp", name="ptmp")
                    nc.vector.scalar_tensor_tensor(
                        ptmp, p_sb[:, 0, :], 1.0, p_sb[:, 1, :],
                        mybir.AluOpType.mult, mybir.AluOpType.add)
                    part = cs_parts[njp // (NPR // 2)]
                    if njp % (NPR // 2) == 0:
                        nc.vector.tensor_copy(part, ptmp)
                    else:
                        nc.vector.tensor_add(part, part, ptmp)
                    if njp >= AHEADP:
                        emit_u_pair(njp - AHEADP)
                    if pending is not None and njp in TAIL_SLOTS:
                        pending[TAIL_SLOTS[njp]]()
                for njp in range(NPR - AHEADP, NPR):
                    emit_u_pair(njp)
                if not is_last:
                    pending = make_tail(msl, u_ps0, u_ps1, cs_parts)
                    continue
                # final m-chunk: the tail has no following A-phase to hide
                # in, so pipeline it against a split u1 chain (u0 finished
                # above): colsum/recip/u0-copy/gd0 overlap u1's 32 MMs.
                cs_ps = psO.tile([1, 512], F32, tag="o")
                nc.tensor.matmul(cs_ps, ones_col, cs_parts[0],
                                 start=True, stop=False)
                nc.tensor.matmul(cs_ps, ones_col, cs_parts[1],
                                 start=False, stop=True)
                recip = small.tile([1, 512], F32R, tag="recip", name="recipf")
                nc.vector.reciprocal(recip, cs_ps)
                u_sb0 = upool.tile([128, 512], BF16, tag="usb0", name="usb0f")
                nc.vector.tensor_copy(u_sb0, u_ps0)
                gd_ps = psO.tile([1, 512], F32, tag="o", name="gdpsf")
                for nj in range(NCH):
                    pj = p_tiles[nj // 2][:, nj % 2, :]
                    nc.tensor.matmul(u_ps1, xT_sb[:, nj, 128:256], pj,
                                     start=(nj == 0), stop=(nj == NCH - 1))
                    if nj == 16:
                        nc.tensor.matmul(gd_ps, gw_sb[:, 0, :], u_sb0,
                                         start=True, stop=False)
                u_sb1 = upool.tile([128, 512], BF16, tag="usb1", name="usb1f")
                nc.vector.tensor_copy(u_sb1, u_ps1)
                # y-term output MMs issue now (gate-independent); they use
                # the psA banks, idle after the last A-phase
                o_pss = []
                for oj in range(2):
                    osl = slice(oj * 128, (oj + 1) * 128)
                    o_ps = psA.tile([128, 512], F32, tag="a", name="ofin")
                    o_pss.append(o_ps)
                    for ci in range(2):
                        nc.tensor.matmul(o_ps, wbt_sb[:, ci, osl],
                                         y_sb[:, ci, msl],
                                         start=(ci == 0), stop=False)
                nc.tensor.matmul(gd_ps, gw_sb[:, 1, :], u_sb1,
                                 start=False, stop=True)
                t_sb = small.tile([1, 512], F32R, tag="t")
                nc.vector.tensor_mul(t_sb, gd_ps, recip)
                th_sb = small.tile([1, 512], F32, tag="e")
                nc.scalar.activation(th_sb, t_sb, Tanh, bias=zero1, scale=0.5)
                scale_sb = small.tile([1, 512], F32R, tag="scale")
                nc.vector.scalar_tensor_tensor(
                    scale_sb, th_sb, 1.0, recip,
                    mybir.AluOpType.add, mybir.AluOpType.mult)
                bc_ps = psO.tile([128, 512], F32, tag="o")
                nc.tensor.matmul(bc_ps, ones_row, scale_sb)
                gated0 = upool.tile([128, 512], BF16, tag="gated0", name="g0f")
                gated1 = upool.tile([128, 512], BF16, tag="gated1", name="g1f")
                nc.vector.tensor_mul(gated0, u_sb0, bc_ps)
                nc.vector.tensor_mul(gated1, u_sb1, bc_ps)
                gated = [gated0, gated1]
                for oj in range(2):
                    osl = slice(oj * 128, (oj + 1) * 128)
                    for ci in range(2):
                        nc.tensor.matmul(o_pss[oj], wat_sb[:, ci, osl],
                                         gated[ci],
                                         start=False, stop=(ci == 1))
                    o_sb = opool.tile([128, 512], F32, tag="osb")
                    nc.vector.tensor_copy(o_sb, o_pss[oj])
                    nc.sync.dma_start(out=OUT[osl, msl], in_=o_sb)
                pending = None
            assert pending is None

    nc.compile()
    return nc


def _get_compiled(n_pix, m_pix, rep=1):
    key = (n_pix, m_pix, rep)
    if key not in _COMPILED:
        _COMPILED[key] = _build_nc(n_pix, m_pix, rep)
    return _COMPILED[key]


def _in_maps(input_1, input_2, W_e, gate_w, W1, W2):
    ex = np.ascontiguousarray(input_1.reshape(B, C, HW), dtype=np.float32)
    q = np.ascontiguousarray(input_2.reshape(B, C, HW), dtype=np.float32)
    W_e = np.asarray(W_e, dtype=np.float32)
    gate_w = np.asarray(gate_w, dtype=np.float32).reshape(C, 1)
    W1 = np.asarray(W1, dtype=np.float32)
    W2 = np.asarray(W2, dtype=np.float32)

    bf = ml_dtypes.bfloat16

    def cb(a):  # contiguous bf16
        return np.ascontiguousarray(np.asarray(a).astype(bf))

    onescol_bf = np.ones((128, 1), bf)
    onesrow = np.full((1, 128), 0.5, np.float32)
    gw_bf = np.ascontiguousarray(gate_w.astype(bf))
    maps = []
    for b in range(B):
        # role Q -> out2[b]
        maps.append({
            "x": cb(ex[b]), "xt": cb(ex[b].T), "y": cb(q[b]),
            "wht": cb(W_e.T),
            "wat": cb(W2[:, :C].T), "wbt": cb(W2[:, C:].T),
            "gw": gw_bf, "onescol": onescol_bf, "onesrow": onesrow,
        })
        # role E -> out1[b]
        maps.append({
            "x": cb(q[b]), "xt": cb(q[b].T), "y": cb(ex[b]),
            "wht": cb(W_e),
            "wat": cb(W1[:, :C].T), "wbt": cb(W1[:, C:].T),
            "gw": gw_bf, "onescol": onescol_bf, "onesrow": onesrow,
        })
    return maps


def kernel(input_1, input_2, W_e, gate_w, W1, W2):
    nc = _get_compiled(HW, HW)
    maps = _in_maps(input_1, input_2, W_e, gate_w, W1, W2)
    res = bass_utils.run_bass_kernel_spmd(
        nc, maps, core_ids=list(range(8)), trace=TRACE
    )
    kernel.last_results = res
    out1 = np.stack([res.results[2 * b + 1]["out"] for b in range(B)])
    out2 = np.stack([res.results[2 * b]["out"] for b in range(B)])
    return out1.reshape(B, C, H, W), out2.reshape(B, C, H, W)
